# revision 3
# baseline (speedup 1.0000x reference)
"""Trainium2 Bass kernel for nn_LCNLinear (locally-connected linear layer).

Reference computation:
    a = zeros(4352*4352); a[idx] = weight; a = a.reshape(4352, 4352)
    y = x @ a.T + bias

Structure exploited: idx comes from np.tile(mask17x17, (256, 256)) row-major
flatnonzero, so the scattered matrix a satisfies
    a[p*17+q, s*17+t] = weight[nnzmask*256*p + 256*pre[q] + bw[q]*s + pos[q,t]]
for mask[q, t] != 0 (zero elsewhere), where bw[q] = row nnz of the mask,
pre[q] = exclusive prefix sum of bw, pos[q,t] = rank of t within row q's
band. The scatter therefore dissolves into strided views of the weight
vector, and y decomposes into 79 dense 256x256x256 block matmuls
    Y[b, p, q] = sum_{t in band(q)} x[b, s, t] @ A3T[q,t][s, p] + bias
with A3T[q,t] a strided view of weight. No scatter is ever materialized.

Precision: operands are split on the host into fp16 hi + lo halves
(v = hi + lo exactly, |lo| <= 2^-11 |v|). The device computes
hi*hi, hi*lo and lo*hi products on the PE at full (1 cycle/row) rate with
fp32 PSUM accumulation; the dropped lo*lo term is O(2^-22). Measured
end-to-end error ~6e-7 — fp32-equivalent — at 1/4 the PE cost of native
fp32 matmuls.

Sharding (8 cores, SPMD single program): output blocks are split into two
p-halves -> 34 (q, ph) units. Each core runs an IDENTICAL schedule of
5 units x 5 band-slots x 2 K-chunks; per-core variation lives only in the
data (which weight/bias slices and which x t-columns the host stages).
Units with bw < 5 / cores with < 5 real units are padded with zero weight
blocks. Per-core x^T tiles are deduplicated into a (2*WSPAN+1)-slot
window shared by the units. Bias is added on-device (DVE
scalar_tensor_tensor, which also combines the hi*hi and hi*lo+lo*hi PSUM
halves), and the per-core Y^T[p, b] outputs are gathered/transposed on
the host.

The host does layout only (shard slicing / transposition / fp16 split);
all FLOPs and the bias add run on the NeuronCores. If idx is NOT a
tiled-mask pattern (it always is for this module), a numpy fallback
computes the reference math directly.
"""

import sys

for _p in ("/opt/trn_rl_repo",):
    if _p not in sys.path:
        sys.path.append(_p)

import numpy as np

SPA = 17
C = 256
B = 256
IN = SPA * C
OUT = SPA * C
NCORES = 8
KC = 2  # K chunks of 128 (C = 256)

_CACHE = {}

# set by test harness to collect profiling info
TRACE = False
LAST_EXEC_TIME_NS = None
LAST_RESULT = None


def _recover_mask(idx):
    """If idx == flatnonzero(tile(mask, (C, C))) for a 17x17 mask, return the
    boolean mask, else None."""
    idx = np.asarray(idx)
    if idx.ndim != 1 or idx.size == 0 or idx.size % (C * C) != 0:
        return None
    nnzmask = idx.size // (C * C)
    if not 1 <= nnzmask <= SPA * SPA:
        return None
    if idx.min() < 0 or idx.max() >= OUT * IN:
        return None
    q = (idx // IN) % SPA
    t = (idx % IN) % SPA
    mask = np.zeros((SPA, SPA), dtype=bool)
    mask[q, t] = True
    if int(mask.sum()) != nnzmask:
        return None
    idx_rec = np.flatnonzero(np.tile(mask, (C, C)))
    if idx_rec.size != idx.size or not np.array_equal(idx, idx_rec.astype(idx.dtype)):
        return None
    return mask


def _schedule(mask):
    """Uniform SPMD schedule: per core [(qA,0),(qA,1),(qB,0),(qB,1), extra]."""
    bw = mask.sum(1).astype(int)
    pre = np.concatenate([[0], np.cumsum(bw)[:-1]]).astype(int)
    nnzmask = int(bw.sum())

    # relative band window: offsets t-q present anywhere in the mask
    qs, ts = np.nonzero(mask)
    rel = ts - qs
    minR, maxR = (int(rel.min()), int(rel.max())) if rel.size else (0, 0)
    WSPAN = maxR - minR + 1  # 5 for the bw=2 band

    UNITS = 5

    # core i -> qA=2i, qB=2i+1 (covers q0..15); leftover q units round-robin
    per_core_q = [[2 * i, 2 * i + 1] for i in range(NCORES)]
    per_core_units = []
    for i in range(NCORES):
        qA, qB = per_core_q[i]
        per_core_units.append([(qA, 0), (qA, 1), (qB, 0), (qB, 1)])
    leftovers = [(qq, ph) for qq in range(16, SPA) for ph in range(2)]
    ci = 0
    for u in leftovers:
        while len(per_core_units[ci]) >= UNITS:
            ci = (ci + 1) % NCORES
        per_core_units[ci].append(u)
        ci = (ci + 1) % NCORES
    for i in range(NCORES):
        per_core_units[i] += [None] * (UNITS - len(per_core_units[i]))

    # second window spans from qC+minR to the furthest band member of any
    # 5th-slot unit's q
    WB = 1
    for i in range(NCORES):
        u4 = per_core_units[i][4]
        if u4 is not None:
            band = np.flatnonzero(mask[u4[0]])
            if band.size:
                WB = max(WB, int(band.max()) - u4[0] - minR + 1)
    NSLOT = (WSPAN + 1) + WB

    # X slot windows per core: slots 0..WSPAN  -> t = qA+minR .. qA+1+maxR
    #                          slots WSPAN+1.. -> t = qC+minR .. qC+minR+WB-1
    # unit u in {0,1}: slot w            (q=qA)
    # unit u in {2,3}: slot w+1          (q=qB=qA+1)
    # unit 4:          slot WSPAN+1+min(w, WB-1)  (q=qC; w>=WB has zero W)
    def slot_of(u, w):
        if u < 2:
            return w
        if u < 4:
            return w + 1
        return WSPAN + 1 + min(w, WB - 1)

    def slot_t(core, si):
        qA = per_core_q[core][0]
        if si <= WSPAN:
            t = qA + minR + si
        else:
            u4 = per_core_units[core][4]
            if u4 is None:
                return None
            t = u4[0] + minR + (si - WSPAN - 1)
        return t if 0 <= t < SPA else None

    return {
        "bw": bw, "pre": pre, "nnzmask": nnzmask, "mask": mask,
        "minR": minR, "WSPAN": WSPAN, "UNITS": UNITS, "NSLOT": NSLOT,
        "WB": WB, "slot_of": slot_of, "units": per_core_units,
        "slot_t": slot_t,
    }


def _build_program(sched):
    import concourse.tile as tile
    from concourse import bacc, mybir

    WSPAN, UNITS, NSLOT = sched["WSPAN"], sched["UNITS"], sched["NSLOT"]
    slot_of = sched["slot_of"]

    nc = bacc.Bacc("TRN2", target_bir_lowering=False, debug=False,
                   num_devices=NCORES)
    # X: [s 128][slot][c][hi|lo 2*B] fp16 (partition-major for big DMAs)
    Xd = nc.dram_tensor("Xc", [128, NSLOT * KC * 2 * B], mybir.dt.float16,
                        kind="ExternalInput").ap()
    # W: [unit][s 128][w][c][hi|lo][p 128] fp16
    Wd = nc.dram_tensor("Wc", [UNITS, 128, WSPAN * KC * 2 * 128],
                        mybir.dt.float16, kind="ExternalInput").ap()
    Bd = nc.dram_tensor("Bc", [128, UNITS], mybir.dt.float32,
                        kind="ExternalInput").ap()
    Yd = nc.dram_tensor("Yc", [128, UNITS * B], mybir.dt.float32,
                        kind="ExternalOutput").ap()

    with tile.TileContext(nc) as tc:
        with (
            tc.tile_pool(name="xp", bufs=1) as xp,
            tc.tile_pool(name="wp", bufs=1) as wp,
            tc.tile_pool(name="bp", bufs=1) as bp,
            tc.tile_pool(name="op", bufs=1) as op,
            tc.tile_pool(name="pp", bufs=4, space="PSUM") as pp,
        ):
            xt = xp.tile([128, NSLOT, KC, 2 * B], mybir.dt.float16)
            wt = wp.tile([128, UNITS, WSPAN, KC, 2, 128], mybir.dt.float16)
            bt = bp.tile([128, UNITS], mybir.dt.float32)
            ot = op.tile([128, UNITS, B], mybir.dt.float32)

            Xd4 = Xd.rearrange("p (s c z) -> p s c z", s=NSLOT, c=KC)

            def load_x(s0, s1):
                # X slot range in one DMA on the SP HWDGE ring
                nc.sync.dma_start(xt[:, s0:s1], Xd4[:, s0:s1])

            def load_w(u0, u1):
                # W unit range on the ACT HWDGE ring (parallel FIFO to SP's)
                nc.scalar.dma_start(
                    wt[:, u0:u1],
                    Wd[u0:u1].rearrange("u p (w c h m) -> p u w c h m",
                                        w=WSPAN, c=KC, h=2))

            def compute(u):
                ps = pp.tile([128, 2 * B], mybir.dt.float32, tag="ps")
                n = WSPAN * KC
                k = 0
                for w in range(WSPAN):
                    si = slot_of(u, w)
                    for c in range(KC):
                        last = k == n - 1
                        # lo x x_hi accumulates into cols 256:512; for the
                        # final block it is emitted first so the group is
                        # closed by a full-bank-span matmul (stop=True must
                        # cover the whole accumulation region).
                        if last:
                            nc.tensor.matmul(
                                ps[:, B:], wt[:, u, w, c, 1, :],
                                xt[:, si, c, :B], start=False, stop=False)
                        # hi x (x_hi | x_lo): cols 0:256 = hh, 256:512 = hl
                        nc.tensor.matmul(
                            ps[:], wt[:, u, w, c, 0, :], xt[:, si, c, :],
                            start=(k == 0), stop=last)
                        if not last:
                            nc.tensor.matmul(
                                ps[:, B:], wt[:, u, w, c, 1, :],
                                xt[:, si, c, :B], start=False, stop=False)
                        k += 1
                # out = (hh + bias) + (hl + lh); DVE may read only one
                # PSUM operand per instruction, so two passes
                nc.vector.tensor_scalar_add(ot[:, u], ps[:, :B], bt[:, u:u + 1])
                nc.vector.tensor_add(ot[:, u], ot[:, u], ps[:, B:])
                nc.sync.dma_start(Yd[:, u * B:(u + 1) * B], ot[:, u])

            # interleave loads with compute so the PE starts as soon as
            # unit 0's operands land
            nc.sync.dma_start(bt[:], Bd[:])
            load_w(0, 1)
            load_x(0, WSPAN + 1)
            load_w(1, 3)
            compute(0)
            load_x(WSPAN + 1, NSLOT)
            load_w(3, 5)
            compute(1)
            compute(2)
            compute(3)
            compute(4)
    nc.compile()
    return nc


def _prep_inputs(x, weight, bias, sched):
    WSPAN, UNITS, NSLOT = sched["WSPAN"], sched["UNITS"], sched["NSLOT"]
    bw, pre, nnzmask = sched["bw"], sched["pre"], sched["nnzmask"]
    mask, minR = sched["mask"], sched["minR"]

    xh = x.astype(np.float16)
    xl = (x - xh.astype(np.float32)).astype(np.float16)
    # [s, t, b] views
    xhT = np.ascontiguousarray(xh.reshape(B, C, SPA).transpose(1, 2, 0))
    xlT = np.ascontiguousarray(xl.reshape(B, C, SPA).transpose(1, 2, 0))

    wh = weight.astype(np.float16)
    wl = (weight - wh.astype(np.float32)).astype(np.float16)

    def a3t_block(src, q, t, ph, c):
        """[128 s, 128 p] strided view of weight array src for block (q,t)."""
        pos = int(np.flatnonzero(mask[q]).tolist().index(t))
        es = src.strides[0]
        view = np.lib.stride_tricks.as_strided(
            src[C * pre[q] + pos:], shape=(C, C),
            strides=(es * int(bw[q]), es * nnzmask * C))
        return view[c * 128:(c + 1) * 128, ph * 128:(ph + 1) * 128]

    in_maps = []
    for core in range(NCORES):
        Xc = np.zeros((128, NSLOT, KC, 2 * B), dtype=np.float16)
        for si in range(NSLOT):
            t = sched["slot_t"](core, si)
            if t is None:
                continue
            for c in range(KC):
                Xc[:, si, c, :B] = xhT[c * 128:(c + 1) * 128, t, :]
                Xc[:, si, c, B:] = xlT[c * 128:(c + 1) * 128, t, :]
        Wc = np.zeros((UNITS, 128, WSPAN, KC, 2, 128), dtype=np.float16)
        Bc = np.zeros((128, UNITS), dtype=np.float32)
        for u, unit in enumerate(sched["units"][core]):
            if unit is None:
                continue
            q, ph = unit
            for w in range(WSPAN):
                t = q + minR + w
                if not (0 <= t < SPA) or not mask[q, t]:
                    continue
                for c in range(KC):
                    Wc[u, :, w, c, 0, :] = a3t_block(wh, q, t, ph, c)
                    Wc[u, :, w, c, 1, :] = a3t_block(wl, q, t, ph, c)
            Bc[:, u] = bias[(ph * 128 + np.arange(128)) * SPA + q]
        in_maps.append({
            "Xc": np.ascontiguousarray(Xc.reshape(NSLOT, 128, KC * 2 * B)),
            "Wc": np.ascontiguousarray(
                Wc.reshape(UNITS, 128, WSPAN * KC * 2 * 128)),
            "Bc": Bc,
        })
    return in_maps


def _gather_output(results, sched):
    y = np.empty((B, C, SPA), dtype=np.float32)
    for core in range(NCORES):
        Yc = results[core]["Yc"].reshape(128, sched["UNITS"], B)
        for u, unit in enumerate(sched["units"][core]):
            if unit is None:
                continue
            q, ph = unit
            y[:, ph * 128:(ph + 1) * 128, q] = Yc[:, u, :].T
    return y.reshape(B, OUT)


def _fallback(x, weight, bias, idx):
    a = np.zeros(OUT * IN, dtype=np.float32)
    a[np.asarray(idx, dtype=np.int64)] = weight
    a = a.reshape(OUT, IN)
    return (x @ a.T + bias).astype(np.float32)


def kernel(x, weight, bias, idx):
    global LAST_EXEC_TIME_NS, LAST_RESULT
    x = np.asarray(x, dtype=np.float32)
    weight = np.asarray(weight, dtype=np.float32)
    bias = np.asarray(bias, dtype=np.float32)
    idx = np.asarray(idx)

    mask = _recover_mask(idx)
    if (mask is None or x.shape != (B, IN) or weight.size != mask.sum() * C * C
            or bias.size != OUT):
        return _fallback(x, weight, bias, idx)

    key = mask.tobytes()
    if key not in _CACHE:
        sched = _schedule(mask)
        nc = _build_program(sched)
        _CACHE[key] = (sched, nc)
    sched, nc = _CACHE[key]

    from concourse.bass_utils import run_bass_kernel_spmd

    in_maps = _prep_inputs(x, weight, bias, sched)
    kwargs = {}
    if TRACE:
        try:
            import profile_hook
            profile_hook.install()
            kwargs["trace"] = True
        except Exception:
            pass
    res = run_bass_kernel_spmd(nc, in_maps, list(range(NCORES)), **kwargs)
    LAST_EXEC_TIME_NS = res.exec_time_ns
    LAST_RESULT = res
    return _gather_output(res.results, sched)



# revision 5
# speedup vs baseline: 1.3422x; 1.3422x over previous
"""Trainium2 Bass kernel for nn_LCNLinear (locally-connected linear layer).

Reference computation:
    a = zeros(4352*4352); a[idx] = weight; a = a.reshape(4352, 4352)
    y = x @ a.T + bias

Structure exploited: idx comes from np.tile(mask17x17, (256, 256)) row-major
flatnonzero, so the scattered matrix dissolves into strided views of the
weight vector and y decomposes into 79 dense 256x256x256 block matmuls
    Y[b, p, q] = sum_{t in band(q)} x[b, s, t] @ A3T[q,t][s, p] + bias
with A3T[q,t] a strided view of weight. No scatter is ever materialized.

Precision: fp16 operands with fp32 PSUM accumulation. Products of two
fp16-rounded operands accumulated in fp32 give ~3e-4 max relative error
(vs the 2e-2 harness gate) at full 1-col/cycle PE rate, 3x less PE work
and ~2x less HBM traffic than a hi+lo split. Outputs are stored fp16
(adds <5e-4) and widened to fp32 on the host.

Sharding (8 cores, SPMD single program): the 34 (q, ph) output groups
(ph = 128-row half of the channel dim) are distributed so each core owns
4-5 groups over a contiguous-ish joint window. Every core runs an
IDENTICAL schedule of 5 PSUM windows sized [5,5,5,5,3] = 23 block-slots;
per-core variation lives only in the data (which weight blocks / which x
t-columns the host stages; unused slots get zero weights). Bias is added
on-device by the DVE copy out of PSUM. A short burst of dummy matmuls on
scratch SBUF runs while the first operands stream in, so the PE's HAM
clock gate is already warm (2.4 GHz) when real work starts.

The host does layout only (shard slicing / transposition / fp16 cast);
all FLOPs and the bias add run on the NeuronCores. If idx is NOT the
bandwidth-2 tiled-mask pattern (it always is for this module), a numpy
fallback computes the reference math directly.
"""

import sys

for _p in ("/opt/trn_rl_repo",):
    if _p not in sys.path:
        sys.path.append(_p)

import numpy as np

SPA = 17
C = 256
B = 256
IN = SPA * C
OUT = SPA * C
NCORES = 8
KC = 2                      # contract chunks of 128 (C = 256)
BW = 2                      # band half-width of the mask
UNITS = 5                   # PSUM windows per core
WIN = [5, 5, 5, 5, 3]       # slots per window
OFF = [0, 5, 10, 15, 20]    # slot offset of each window
BASES = [0, 0, 1, 1, 5]     # X-slot base of each window
SL = 23                     # total W slots per core
NSLOT = 8                   # X t-slots per core

_CACHE = {}

# set by test harness to collect profiling info
TRACE = False
LAST_EXEC_TIME_NS = None
LAST_RESULT = None


def _recover_mask(idx):
    """If idx == flatnonzero(tile(mask, (C, C))) for a 17x17 mask, return the
    boolean mask, else None."""
    idx = np.asarray(idx)
    if idx.ndim != 1 or idx.size == 0 or idx.size % (C * C) != 0:
        return None
    nnzmask = idx.size // (C * C)
    if not 1 <= nnzmask <= SPA * SPA:
        return None
    if idx.min() < 0 or idx.max() >= OUT * IN:
        return None
    q = (idx // IN) % SPA
    t = (idx % IN) % SPA
    mask = np.zeros((SPA, SPA), dtype=bool)
    mask[q, t] = True
    if int(mask.sum()) != nnzmask:
        return None
    idx_rec = np.flatnonzero(np.tile(mask, (C, C)))
    if idx_rec.size != idx.size or not np.array_equal(idx, idx_rec.astype(idx.dtype)):
        return None
    return mask


def _schedule(mask):
    """Static schedule for the |i-j|<=BW band mask. Returns None if the mask
    is not that band (callers then fall back to numpy)."""
    i = np.arange(SPA)
    band = np.abs(i[:, None] - i[None, :]) <= BW
    if not np.array_equal(mask, band):
        return None

    units = []
    slot_t = []
    for core in range(6):          # cores 0..5: q = 2+2c, 3+2c
        qA = 2 + 2 * core
        u = [(qA, 0), (qA, 1), (qA + 1, 0), (qA + 1, 1), None]
        st = [qA - 2, qA - 1, qA, qA + 1, qA + 2, qA + 3, None, None]
        units.append(u)
        slot_t.append(st)
    # core 5 additionally owns (16, 0) in its 5th window
    units[5][4] = (16, 0)
    slot_t[5][6] = 16
    slot_t[5][7] = 14
    # core 6: q14, q15
    units.append([(14, 0), (14, 1), (15, 0), (15, 1), None])
    slot_t.append([12, 13, 14, 15, 16, None, None, None])
    # core 7: q0, q1, (16,1)
    units.append([(0, 0), (0, 1), (1, 0), (1, 1), (16, 1)])
    slot_t.append([0, 1, 2, 3, 0, 14, 15, 16])

    bw = mask.sum(1).astype(int)
    pre = np.concatenate([[0], np.cumsum(bw)[:-1]]).astype(int)
    return {
        "mask": mask, "bw": bw, "pre": pre, "nnzmask": int(bw.sum()),
        "units": units, "slot_t": slot_t,
    }


def _build_program():
    import concourse.tile as tile
    from concourse import bacc, mybir

    f16, f32 = mybir.dt.float16, mybir.dt.float32
    nc = bacc.Bacc("TRN2", target_bir_lowering=False, debug=False,
                   num_devices=NCORES)
    # X: [s 128][slot][c][b] fp16, W: [s 128][slot][c][p 128] fp16
    Xd = nc.dram_tensor("Xc", [128, NSLOT * KC * B], f16,
                        kind="ExternalInput").ap()
    Wd = nc.dram_tensor("Wc", [128, SL * KC * 128], f16,
                        kind="ExternalInput").ap()
    Bd = nc.dram_tensor("Bc", [128, UNITS], f32, kind="ExternalInput").ap()
    Yd = nc.dram_tensor("Yc", [128, UNITS * B], f16,
                        kind="ExternalOutput").ap()

    with tile.TileContext(nc) as tc:
        with (
            tc.tile_pool(name="xp", bufs=1) as xp,
            tc.tile_pool(name="wp", bufs=1) as wp,
            tc.tile_pool(name="bp", bufs=1) as bp,
            tc.tile_pool(name="op", bufs=1) as op,
            tc.tile_pool(name="wu", bufs=1) as wu,
            tc.tile_pool(name="pp", bufs=1, space="PSUM") as pp,
        ):
            xt = xp.tile([128, NSLOT, KC, B], f16)
            wt = wp.tile([128, SL, KC, 128], f16)
            bt = bp.tile([128, UNITS], f32)
            ot = op.tile([128, UNITS, B], f16)
            dw = wu.tile([128, 128], f16)
            dx = wu.tile([128, 512], f16)

            Xd4 = Xd.rearrange("p (s c b) -> p s c b", s=NSLOT, c=KC)
            Wd4 = Wd.rearrange("p (s c m) -> p s c m", s=SL, c=KC)

            # PE warm-up: dummy matmuls on zeroed scratch trip the HAM
            # activity window while the real operands stream in, so the
            # first real matmul already runs at 2.4 GHz.
            nc.gpsimd.memset(dw[:], 0)
            nc.gpsimd.memset(dx[:], 0)
            pw = pp.tile([128, 512], f32, tag="warm")
            for _ in range(10):
                nc.tensor.matmul(pw[:], dw[:], dx[:], start=True, stop=True)

            # W on the ACT HWDGE ring; X + bias + Y on the SP ring.
            nc.scalar.dma_start(wt[:, 0:5], Wd4[:, 0:5])
            nc.sync.dma_start(xt[:, 0:5], Xd4[:, 0:5])
            nc.scalar.dma_start(wt[:, 5:SL], Wd4[:, 5:SL])
            nc.sync.dma_start(xt[:, 5:NSLOT], Xd4[:, 5:NSLOT])
            nc.sync.dma_start(bt[:], Bd)

            for u in range(UNITS):
                ps = pp.tile([128, B], f32, tag=f"ps{u}")
                n = WIN[u] * KC
                k = 0
                for w in range(WIN[u]):
                    si = BASES[u] + w
                    slot = OFF[u] + w
                    for c in range(KC):
                        nc.tensor.matmul(ps[:], wt[:, slot, c, :],
                                         xt[:, si, c, :],
                                         start=(k == 0), stop=(k == n - 1))
                        k += 1
                # out = psum + bias (DVE reads PSUM, writes fp16 SBUF)
                nc.vector.tensor_scalar_add(ot[:, u], ps[:], bt[:, u:u + 1])
                nc.sync.dma_start(Yd[:, u * B:(u + 1) * B], ot[:, u])
    nc.compile()
    return nc


def _prep_inputs(x, weight, bias, sched):
    mask, bw, pre = sched["mask"], sched["bw"], sched["pre"]
    nnzmask = sched["nnzmask"]

    xh = x.astype(np.float16)
    wh = weight.astype(np.float16)
    # [c, t, b] view of x
    xhT = np.ascontiguousarray(xh.reshape(B, C, SPA).transpose(1, 2, 0))

    def a3t_block(src, q, t, ph, c):
        """[128 s, 128 p] strided view of weight array src for block (q,t)."""
        pos = int(np.flatnonzero(mask[q]).tolist().index(t))
        es = src.strides[0]
        view = np.lib.stride_tricks.as_strided(
            src[C * pre[q] + pos:], shape=(C, C),
            strides=(es * int(bw[q]), es * nnzmask * C))
        return view[c * 128:(c + 1) * 128, ph * 128:(ph + 1) * 128]

    in_maps = []
    for core in range(NCORES):
        slot_t = sched["slot_t"][core]
        Xc = np.zeros((128, NSLOT, KC, B), dtype=np.float16)
        for si, t in enumerate(slot_t):
            if t is None:
                continue
            for c in range(KC):
                Xc[:, si, c, :] = xhT[c * 128:(c + 1) * 128, t, :]
        Wc = np.zeros((128, SL, KC, 128), dtype=np.float16)
        Bc = np.zeros((128, UNITS), dtype=np.float32)
        for u, unit in enumerate(sched["units"][core]):
            if unit is None:
                continue
            q, ph = unit
            needed = set(np.flatnonzero(mask[q]).tolist())
            for w in range(WIN[u]):
                si = BASES[u] + w
                t = slot_t[si] if si < NSLOT else None
                if t is not None and t in needed:
                    needed.discard(t)
                    for c in range(KC):
                        Wc[:, OFF[u] + w, c, :] = a3t_block(wh, q, t, ph, c)
            assert not needed, (core, u, unit, needed)
            Bc[:, u] = bias[(ph * 128 + np.arange(128)) * SPA + q]
        in_maps.append({
            "Xc": np.ascontiguousarray(Xc.reshape(128, -1)),
            "Wc": np.ascontiguousarray(Wc.reshape(128, -1)),
            "Bc": Bc,
        })
    return in_maps


def _gather_output(results, sched):
    y = np.zeros((B, C, SPA), dtype=np.float32)
    for core in range(NCORES):
        Yc = results[core]["Yc"].reshape(128, UNITS, B)
        for u, unit in enumerate(sched["units"][core]):
            if unit is None:
                continue
            q, ph = unit
            y[:, ph * 128:(ph + 1) * 128, q] = Yc[:, u, :].T.astype(np.float32)
    return y.reshape(B, OUT)


def _fallback(x, weight, bias, idx):
    a = np.zeros(OUT * IN, dtype=np.float32)
    a[np.asarray(idx, dtype=np.int64)] = weight
    a = a.reshape(OUT, IN)
    return (x @ a.T + bias).astype(np.float32)


def kernel(x, weight, bias, idx):
    global LAST_EXEC_TIME_NS, LAST_RESULT
    x = np.asarray(x, dtype=np.float32)
    weight = np.asarray(weight, dtype=np.float32)
    bias = np.asarray(bias, dtype=np.float32)
    idx = np.asarray(idx)

    mask = _recover_mask(idx)
    sched = None
    if (mask is not None and x.shape == (B, IN)
            and weight.size == mask.sum() * C * C and bias.size == OUT):
        sched = _schedule(mask)
    if sched is None:
        return _fallback(x, weight, bias, idx)

    key = mask.tobytes()
    if key not in _CACHE:
        _CACHE[key] = (sched, _build_program())
    sched, nc = _CACHE[key]

    from concourse.bass_utils import run_bass_kernel_spmd

    in_maps = _prep_inputs(x, weight, bias, sched)
    kwargs = {}
    if TRACE:
        try:
            import profile_hook
            profile_hook.install()
            kwargs["trace"] = True
        except Exception:
            pass
    res = run_bass_kernel_spmd(nc, in_maps, list(range(NCORES)), **kwargs)
    LAST_EXEC_TIME_NS = res.exec_time_ns
    LAST_RESULT = res
    return _gather_output(res.results, sched)


# revision 8
# speedup vs baseline: 1.6119x; 1.2009x over previous
"""Trainium2 Bass kernel for nn_LCNLinear (locally-connected linear layer).

Reference computation:
    a = zeros(4352*4352); a[idx] = weight; a = a.reshape(4352, 4352)
    y = x @ a.T + bias

Structure exploited: idx comes from np.tile(mask17x17, (256, 256)) row-major
flatnonzero, so the scattered matrix dissolves into strided views of the
weight vector and y decomposes into 79 dense 256x256x256 block matmuls
    Y[b, p, q] = sum_{t in band(q)} x[b, s, t] @ A3T[q,t][s, p] + bias
with A3T[q,t] a strided view of weight. No scatter is ever materialized.

Precision: fp16 operands with fp32 PSUM accumulation. Products of two
fp16-rounded operands accumulated in fp32 give ~3e-4 max relative error
(vs the 2e-2 harness gate) at full 1-col/cycle PE rate, 3x less PE work
and ~2x less HBM traffic than a hi+lo split. Outputs are stored fp16
(adds <5e-4) and widened to fp32 on the host.

Sharding (8 cores, SPMD single program): the 34 (q, ph) output groups
(ph = 128-row half of the channel dim) are distributed so each core owns
4-5 groups over a contiguous-ish joint window. Every core runs an
IDENTICAL schedule of 5 PSUM windows sized [5,5,5,5,3] = 23 block-slots;
per-core variation lives only in the data (which weight blocks / which x
t-columns the host stages; unused slots get zero weights). Bias is added
on-device by the DVE copy out of PSUM. A short burst of dummy matmuls on
scratch SBUF runs while the first operands stream in, so the PE's HAM
clock gate is already warm (2.4 GHz) when real work starts.

The host does layout only (shard slicing / transposition / fp16 cast);
all FLOPs and the bias add run on the NeuronCores. If idx is NOT the
bandwidth-2 tiled-mask pattern (it always is for this module), a numpy
fallback computes the reference math directly.
"""

import sys

for _p in ("/opt/trn_rl_repo",):
    if _p not in sys.path:
        sys.path.append(_p)

import numpy as np

SPA = 17
C = 256
B = 256
IN = SPA * C
OUT = SPA * C
NCORES = 8
KC = 2                      # contract chunks of 128 (C = 256)
BW = 2                      # band half-width of the mask
UNITS = 5                   # PSUM windows per core
WIN = [5, 5, 5, 5, 3]       # slots per window
OFF = [0, 5, 10, 15, 20]    # slot offset of each window
BASES = [0, 0, 1, 1, 5]     # X-slot base of each window
SL = 23                     # total W slots per core
NSLOT = 8                   # X t-slots per core

_CACHE = {}

# set by test harness to collect profiling info
TRACE = False
LAST_EXEC_TIME_NS = None
LAST_RESULT = None


def _recover_mask(idx):
    """If idx == flatnonzero(tile(mask, (C, C))) for a 17x17 mask, return the
    boolean mask, else None."""
    idx = np.asarray(idx)
    if idx.ndim != 1 or idx.size == 0 or idx.size % (C * C) != 0:
        return None
    nnzmask = idx.size // (C * C)
    if not 1 <= nnzmask <= SPA * SPA:
        return None
    if idx.min() < 0 or idx.max() >= OUT * IN:
        return None
    q = (idx // IN) % SPA
    t = (idx % IN) % SPA
    mask = np.zeros((SPA, SPA), dtype=bool)
    mask[q, t] = True
    if int(mask.sum()) != nnzmask:
        return None
    idx_rec = np.flatnonzero(np.tile(mask, (C, C)))
    if idx_rec.size != idx.size or not np.array_equal(idx, idx_rec.astype(idx.dtype)):
        return None
    return mask


def _schedule(mask):
    """Static schedule for the |i-j|<=BW band mask. Returns None if the mask
    is not that band (callers then fall back to numpy)."""
    i = np.arange(SPA)
    band = np.abs(i[:, None] - i[None, :]) <= BW
    if not np.array_equal(mask, band):
        return None

    units = []
    slot_t = []
    for core in range(6):          # cores 0..5: q = 2+2c, 3+2c
        qA = 2 + 2 * core
        u = [(qA, 0), (qA, 1), (qA + 1, 0), (qA + 1, 1), None]
        st = [qA - 2, qA - 1, qA, qA + 1, qA + 2, qA + 3, None, None]
        units.append(u)
        slot_t.append(st)
    # core 5 additionally owns (16, 0) in its 5th window
    units[5][4] = (16, 0)
    slot_t[5][6] = 16
    slot_t[5][7] = 14
    # core 6: q14, q15
    units.append([(14, 0), (14, 1), (15, 0), (15, 1), None])
    slot_t.append([12, 13, 14, 15, 16, None, None, None])
    # core 7: q0, q1, (16,1)
    units.append([(0, 0), (0, 1), (1, 0), (1, 1), (16, 1)])
    slot_t.append([0, 1, 2, 3, 0, 14, 15, 16])

    bw = mask.sum(1).astype(int)
    pre = np.concatenate([[0], np.cumsum(bw)[:-1]]).astype(int)
    return {
        "mask": mask, "bw": bw, "pre": pre, "nnzmask": int(bw.sum()),
        "units": units, "slot_t": slot_t,
    }


def _build_program():
    import concourse.tile as tile
    from concourse import bacc, mybir

    f16, f32 = mybir.dt.float16, mybir.dt.float32
    nc = bacc.Bacc("TRN2", target_bir_lowering=False, debug=False,
                   num_devices=NCORES)
    # One DRAM tensor per DMA transfer so each source is a dense block.
    # X pieces: slots [0:5) and [5:8); W pieces: slots [0:5), [5:10),
    # [10:15), [15:23). Layout inside each: [s 128][slot][c][...]
    Xa = nc.dram_tensor("Xa", [128, 5 * KC * B], f16,
                        kind="ExternalInput").ap()
    Xb = nc.dram_tensor("Xb", [128, (NSLOT - 5) * KC * B], f16,
                        kind="ExternalInput").ap()
    Wa = nc.dram_tensor("Wa", [128, 5 * KC * 128], f16,
                        kind="ExternalInput").ap()
    Wb = nc.dram_tensor("Wb", [128, 5 * KC * 128], f16,
                        kind="ExternalInput").ap()
    Wc = nc.dram_tensor("Wc", [128, 5 * KC * 128], f16,
                        kind="ExternalInput").ap()
    Wd = nc.dram_tensor("Wd", [128, 8 * KC * 128], f16,
                        kind="ExternalInput").ap()
    Bd = nc.dram_tensor("Bc", [128, UNITS], f32, kind="ExternalInput").ap()
    Yd = nc.dram_tensor("Yc", [128, UNITS * B], f16,
                        kind="ExternalOutput").ap()

    with tile.TileContext(nc) as tc:
        with (
            tc.tile_pool(name="xp", bufs=1) as xp,
            tc.tile_pool(name="wp", bufs=1) as wp,
            tc.tile_pool(name="bp", bufs=1) as bp,
            tc.tile_pool(name="op", bufs=1) as op,
            tc.tile_pool(name="wu", bufs=1) as wu,
            tc.tile_pool(name="pp", bufs=1, space="PSUM") as pp,
        ):
            xt = xp.tile([128, NSLOT, KC, B], f16)
            wt = wp.tile([128, SL, KC, 128], f16)
            bt = bp.tile([128, UNITS], f32)
            ot = op.tile([128, UNITS, B], f16)
            dw = wu.tile([128, 128], f16)
            dx = wu.tile([128, 512], f16)

            # PE warm-up: dummy matmuls on zeroed scratch trip the HAM
            # activity window while the real operands stream in, so the
            # first real matmul already runs at 2.4 GHz.
            nc.gpsimd.memset(dw[:], 0)
            nc.gpsimd.memset(dx[:], 0)
            pw = pp.tile([128, 512], f32, tag="warm")
            for _ in range(10):
                nc.tensor.matmul(pw[:], dw[:], dx[:], start=True, stop=True)

            # W on the ACT HWDGE ring (window-aligned pieces so each
            # window's matmuls gate only on their own slots); X + bias + Y
            # on the SP ring.
            nc.scalar.dma_start(
                wt[:, 0:5], Wa.rearrange("p (s c m) -> p s c m", s=5, c=KC))
            nc.sync.dma_start(
                xt[:, 0:5], Xa.rearrange("p (s c b) -> p s c b", s=5, c=KC))
            nc.scalar.dma_start(
                wt[:, 5:10], Wb.rearrange("p (s c m) -> p s c m", s=5, c=KC))
            nc.scalar.dma_start(
                wt[:, 10:15], Wc.rearrange("p (s c m) -> p s c m", s=5, c=KC))
            nc.scalar.dma_start(
                wt[:, 15:SL], Wd.rearrange("p (s c m) -> p s c m", s=8, c=KC))
            nc.sync.dma_start(
                xt[:, 5:NSLOT],
                Xb.rearrange("p (s c b) -> p s c b", s=NSLOT - 5, c=KC))
            nc.sync.dma_start(bt[:], Bd)

            for u in range(UNITS):
                ps = pp.tile([128, B], f32, tag=f"ps{u}")
                n = WIN[u] * KC
                k = 0
                for w in range(WIN[u]):
                    si = BASES[u] + w
                    slot = OFF[u] + w
                    for c in range(KC):
                        nc.tensor.matmul(ps[:], wt[:, slot, c, :],
                                         xt[:, si, c, :],
                                         start=(k == 0), stop=(k == n - 1))
                        k += 1
                # out = psum + bias (DVE reads PSUM, writes fp16 SBUF)
                nc.vector.tensor_scalar_add(ot[:, u], ps[:], bt[:, u:u + 1])
                nc.sync.dma_start(Yd[:, u * B:(u + 1) * B], ot[:, u])
    nc.compile()
    return nc


def _prep_inputs(x, weight, bias, sched):
    mask, bw, pre = sched["mask"], sched["bw"], sched["pre"]
    nnzmask = sched["nnzmask"]

    xh = x.astype(np.float16)
    wh = weight.astype(np.float16)
    # [c, t, b] view of x
    xhT = np.ascontiguousarray(xh.reshape(B, C, SPA).transpose(1, 2, 0))

    def a3t_block(src, q, t, ph, c):
        """[128 s, 128 p] strided view of weight array src for block (q,t)."""
        pos = int(np.flatnonzero(mask[q]).tolist().index(t))
        es = src.strides[0]
        view = np.lib.stride_tricks.as_strided(
            src[C * pre[q] + pos:], shape=(C, C),
            strides=(es * int(bw[q]), es * nnzmask * C))
        return view[c * 128:(c + 1) * 128, ph * 128:(ph + 1) * 128]

    in_maps = []
    for core in range(NCORES):
        slot_t = sched["slot_t"][core]
        Xc = np.zeros((128, NSLOT, KC, B), dtype=np.float16)
        for si, t in enumerate(slot_t):
            if t is None:
                continue
            for c in range(KC):
                Xc[:, si, c, :] = xhT[c * 128:(c + 1) * 128, t, :]
        Wc = np.zeros((128, SL, KC, 128), dtype=np.float16)
        Bc = np.zeros((128, UNITS), dtype=np.float32)
        for u, unit in enumerate(sched["units"][core]):
            if unit is None:
                continue
            q, ph = unit
            needed = set(np.flatnonzero(mask[q]).tolist())
            for w in range(WIN[u]):
                si = BASES[u] + w
                t = slot_t[si] if si < NSLOT else None
                if t is not None and t in needed:
                    needed.discard(t)
                    for c in range(KC):
                        Wc[:, OFF[u] + w, c, :] = a3t_block(wh, q, t, ph, c)
            assert not needed, (core, u, unit, needed)
            Bc[:, u] = bias[(ph * 128 + np.arange(128)) * SPA + q]
        Xf = Xc.reshape(128, NSLOT, KC * B)
        Wf = Wc.reshape(128, SL, KC * 128)
        in_maps.append({
            "Xa": np.ascontiguousarray(Xf[:, 0:5].reshape(128, -1)),
            "Xb": np.ascontiguousarray(Xf[:, 5:NSLOT].reshape(128, -1)),
            "Wa": np.ascontiguousarray(Wf[:, 0:5].reshape(128, -1)),
            "Wb": np.ascontiguousarray(Wf[:, 5:10].reshape(128, -1)),
            "Wc": np.ascontiguousarray(Wf[:, 10:15].reshape(128, -1)),
            "Wd": np.ascontiguousarray(Wf[:, 15:SL].reshape(128, -1)),
            "Bc": Bc,
        })
    return in_maps


def _gather_output(results, sched):
    y = np.zeros((B, C, SPA), dtype=np.float32)
    for core in range(NCORES):
        Yc = results[core]["Yc"].reshape(128, UNITS, B)
        for u, unit in enumerate(sched["units"][core]):
            if unit is None:
                continue
            q, ph = unit
            y[:, ph * 128:(ph + 1) * 128, q] = Yc[:, u, :].T.astype(np.float32)
    return y.reshape(B, OUT)


def _fallback(x, weight, bias, idx):
    a = np.zeros(OUT * IN, dtype=np.float32)
    a[np.asarray(idx, dtype=np.int64)] = weight
    a = a.reshape(OUT, IN)
    return (x @ a.T + bias).astype(np.float32)


def kernel(x, weight, bias, idx):
    global LAST_EXEC_TIME_NS, LAST_RESULT
    x = np.asarray(x, dtype=np.float32)
    weight = np.asarray(weight, dtype=np.float32)
    bias = np.asarray(bias, dtype=np.float32)
    idx = np.asarray(idx)

    mask = _recover_mask(idx)
    sched = None
    if (mask is not None and x.shape == (B, IN)
            and weight.size == mask.sum() * C * C and bias.size == OUT):
        sched = _schedule(mask)
    if sched is None:
        return _fallback(x, weight, bias, idx)

    key = mask.tobytes()
    if key not in _CACHE:
        _CACHE[key] = (sched, _build_program())
    sched, nc = _CACHE[key]

    from concourse.bass_utils import run_bass_kernel_spmd

    in_maps = _prep_inputs(x, weight, bias, sched)
    kwargs = {}
    if TRACE:
        try:
            import profile_hook
            profile_hook.install()
            kwargs["trace"] = True
        except Exception:
            pass
    res = run_bass_kernel_spmd(nc, in_maps, list(range(NCORES)), **kwargs)
    LAST_EXEC_TIME_NS = res.exec_time_ns
    LAST_RESULT = res
    return _gather_output(res.results, sched)


# revision 12
# speedup vs baseline: 1.6399x; 1.0173x over previous
"""Trainium2 Bass kernel for nn_LCNLinear (locally-connected linear layer).

Reference computation:
    a = zeros(4352*4352); a[idx] = weight; a = a.reshape(4352, 4352)
    y = x @ a.T + bias

Structure exploited: idx comes from np.tile(mask17x17, (256, 256)) row-major
flatnonzero, so the scattered matrix dissolves into strided views of the
weight vector and y decomposes into 79 dense 256x256x256 block matmuls
    Y[b, p, q] = sum_{t in band(q)} x[b, s, t] @ A3T[q,t][s, p] + bias
with A3T[q,t] a strided view of weight. No scatter is ever materialized.

Precision: fp16 operands with fp32 PSUM accumulation. Products of two
fp16-rounded operands accumulated in fp32 give ~3e-4 max relative error
(vs the 2e-2 harness gate) at full 1-col/cycle PE rate, 3x less PE work
and ~2x less HBM traffic than a hi+lo split. Outputs are stored fp16
(adds <5e-4) and widened to fp32 on the host.

Sharding (8 cores, SPMD single program): the 34 (q, ph) output groups
(ph = 128-row half of the channel dim) are distributed so each core owns
4-5 groups over a contiguous-ish joint window. Every core runs an
IDENTICAL schedule of 5 PSUM windows sized [5,5,5,5,3] = 23 block-slots;
per-core variation lives only in the data (which weight blocks / which x
t-columns the host stages; unused slots get zero weights). Bias is added
on-device by the DVE copy out of PSUM. A short burst of dummy matmuls on
scratch SBUF runs while the first operands stream in, so the PE's HAM
clock gate is already warm (2.4 GHz) when real work starts.

The host does layout only (shard slicing / transposition / fp16 cast);
all FLOPs and the bias add run on the NeuronCores. If idx is NOT the
bandwidth-2 tiled-mask pattern (it always is for this module), a numpy
fallback computes the reference math directly.
"""

import sys

for _p in ("/opt/trn_rl_repo",):
    if _p not in sys.path:
        sys.path.append(_p)

import numpy as np

SPA = 17
C = 256
B = 256
IN = SPA * C
OUT = SPA * C
NCORES = 8
KC = 2                      # contract chunks of 128 (C = 256)
BW = 2                      # band half-width of the mask
UNITS = 5                   # PSUM windows per core
WIN = [5, 5, 5, 5, 3]       # slots per window
OFF = [0, 5, 10, 15, 20]    # slot offset of each window
BASES = [0, 0, 1, 1, 5]     # X-slot base of each window
SL = 23                     # total W slots per core
NSLOT = 8                   # X t-slots per core

_CACHE = {}

# set by test harness to collect profiling info
TRACE = False
LAST_EXEC_TIME_NS = None
LAST_RESULT = None


def _recover_mask(idx):
    """If idx == flatnonzero(tile(mask, (C, C))) for a 17x17 mask, return the
    boolean mask, else None."""
    idx = np.asarray(idx)
    if idx.ndim != 1 or idx.size == 0 or idx.size % (C * C) != 0:
        return None
    nnzmask = idx.size // (C * C)
    if not 1 <= nnzmask <= SPA * SPA:
        return None
    if idx.min() < 0 or idx.max() >= OUT * IN:
        return None
    q = (idx // IN) % SPA
    t = (idx % IN) % SPA
    mask = np.zeros((SPA, SPA), dtype=bool)
    mask[q, t] = True
    if int(mask.sum()) != nnzmask:
        return None
    idx_rec = np.flatnonzero(np.tile(mask, (C, C)))
    if idx_rec.size != idx.size or not np.array_equal(idx, idx_rec.astype(idx.dtype)):
        return None
    return mask


def _schedule(mask):
    """Static schedule for the |i-j|<=BW band mask. Returns None if the mask
    is not that band (callers then fall back to numpy)."""
    i = np.arange(SPA)
    band = np.abs(i[:, None] - i[None, :]) <= BW
    if not np.array_equal(mask, band):
        return None

    units = []
    slot_t = []
    for core in range(6):          # cores 0..5: q = 2+2c, 3+2c
        qA = 2 + 2 * core
        u = [(qA, 0), (qA, 1), (qA + 1, 0), (qA + 1, 1), None]
        st = [qA - 2, qA - 1, qA, qA + 1, qA + 2, qA + 3, None, None]
        units.append(u)
        slot_t.append(st)
    # core 5 additionally owns (16, 0) in its 5th window
    units[5][4] = (16, 0)
    slot_t[5][6] = 16
    slot_t[5][7] = 14
    # core 6: q14, q15
    units.append([(14, 0), (14, 1), (15, 0), (15, 1), None])
    slot_t.append([12, 13, 14, 15, 16, None, None, None])
    # core 7: q0, q1, (16,1)
    units.append([(0, 0), (0, 1), (1, 0), (1, 1), (16, 1)])
    slot_t.append([0, 1, 2, 3, 0, 14, 15, 16])

    bw = mask.sum(1).astype(int)
    pre = np.concatenate([[0], np.cumsum(bw)[:-1]]).astype(int)
    return {
        "mask": mask, "bw": bw, "pre": pre, "nnzmask": int(bw.sum()),
        "units": units, "slot_t": slot_t,
    }


def _build_program():
    import concourse.tile as tile
    from concourse import bacc, mybir

    f16, f32 = mybir.dt.float16, mybir.dt.float32
    nc = bacc.Bacc("TRN2", target_bir_lowering=False, debug=False,
                   num_devices=NCORES)
    # One DRAM tensor per DMA transfer so each source is a dense block.
    # X pieces: slots [0:3), [3:5), [5:8); W pieces: slots [0:5), [5:10),
    # [10:15), [15:20), [20:23). Layout inside each: [s 128][slot][c][...]
    def dram(name, slots, inner):
        return nc.dram_tensor(name, [128, slots * KC * inner], f16,
                              kind="ExternalInput").ap()

    Xa, Xb, Xc = dram("Xa", 3, B), dram("Xb", 2, B), dram("Xc", 3, B)
    Wa, Wb, Wc = dram("Wa", 5, 128), dram("Wb", 5, 128), dram("Wc", 5, 128)
    We, Wf = dram("We", 5, 128), dram("Wf", 3, 128)
    Bd = nc.dram_tensor("Bc", [128, UNITS], f32, kind="ExternalInput").ap()
    Yd = nc.dram_tensor("Yc", [128, UNITS * B], f16,
                        kind="ExternalOutput").ap()

    with tile.TileContext(nc) as tc:
        with (
            tc.tile_pool(name="xp", bufs=1) as xp,
            tc.tile_pool(name="wp", bufs=1) as wp,
            tc.tile_pool(name="bp", bufs=1) as bp,
            tc.tile_pool(name="op", bufs=1) as op,
            tc.tile_pool(name="wu", bufs=1) as wu,
            tc.tile_pool(name="pp", bufs=1, space="PSUM") as pp,
        ):
            xt = xp.tile([128, NSLOT, KC, B], f16)
            wt = wp.tile([128, SL, KC, 128], f16)
            bt = bp.tile([128, UNITS], f32)
            ot = op.tile([128, UNITS, B], f16)
            dw = wu.tile([128, 128], f16)
            dx = wu.tile([128, 512], f16)

            # PE warm-up: dummy matmuls on zeroed scratch trip the HAM
            # activity window while the real operands stream in, so the
            # first real matmul already runs at 2.4 GHz.
            nc.gpsimd.memset(dw[:], 0)
            nc.gpsimd.memset(dx[:], 0)
            pw = pp.tile([128, 512], f32, tag="warm")
            for _ in range(8):
                nc.tensor.matmul(pw[:], dw[:], dx[:], start=True, stop=True)

            # Reads are balanced across both HWDGE rings (~1.3 MB each) in
            # ~0.33 MB pieces ordered by when the PE needs them, so each
            # window's matmuls gate only on their own slots.
            def rx(t, s):
                return t.rearrange("p (s c b) -> p s c b", s=s, c=KC)

            def rw(t, s):
                return t.rearrange("p (s c m) -> p s c m", s=s, c=KC)

            nc.sync.dma_start(xt[:, 0:3], rx(Xa, 3))
            nc.scalar.dma_start(wt[:, 0:5], rw(Wa, 5))
            nc.sync.dma_start(xt[:, 3:5], rx(Xb, 2))
            nc.scalar.dma_start(wt[:, 5:10], rw(Wb, 5))
            nc.sync.dma_start(wt[:, 15:20], rw(We, 5))
            nc.scalar.dma_start(wt[:, 10:15], rw(Wc, 5))
            nc.sync.dma_start(xt[:, 5:NSLOT], rx(Xc, 3))
            nc.sync.dma_start(bt[:], Bd)
            nc.sync.dma_start(wt[:, 20:SL], rw(Wf, 3))

            for u in range(UNITS):
                ps = pp.tile([128, B], f32, tag=f"ps{u}")
                n = WIN[u] * KC
                k = 0
                for w in range(WIN[u]):
                    si = BASES[u] + w
                    slot = OFF[u] + w
                    for c in range(KC):
                        nc.tensor.matmul(ps[:], wt[:, slot, c, :],
                                         xt[:, si, c, :],
                                         start=(k == 0), stop=(k == n - 1))
                        k += 1
                # out = psum + bias (DVE reads PSUM, writes fp16 SBUF)
                nc.vector.tensor_scalar_add(ot[:, u], ps[:], bt[:, u:u + 1])
                # batched stores: windows 0-3 go out as one 2KB-row DMA,
                # the final 3-slot window alone so the tail is short
                if u == 3:
                    nc.sync.dma_start(Yd[:, 0:4 * B], ot[:, 0:4])
                elif u == 4:
                    nc.sync.dma_start(Yd[:, 4 * B:], ot[:, 4])
    nc.compile()
    return nc


def _prep_inputs(x, weight, bias, sched):
    mask, bw, pre = sched["mask"], sched["bw"], sched["pre"]
    nnzmask = sched["nnzmask"]

    xh = x.astype(np.float16)
    wh = weight.astype(np.float16)
    # [c, t, b] view of x
    xhT = np.ascontiguousarray(xh.reshape(B, C, SPA).transpose(1, 2, 0))

    def a3t_block(src, q, t, ph, c):
        """[128 s, 128 p] strided view of weight array src for block (q,t)."""
        pos = int(np.flatnonzero(mask[q]).tolist().index(t))
        es = src.strides[0]
        view = np.lib.stride_tricks.as_strided(
            src[C * pre[q] + pos:], shape=(C, C),
            strides=(es * int(bw[q]), es * nnzmask * C))
        return view[c * 128:(c + 1) * 128, ph * 128:(ph + 1) * 128]

    in_maps = []
    for core in range(NCORES):
        slot_t = sched["slot_t"][core]
        Xc = np.zeros((128, NSLOT, KC, B), dtype=np.float16)
        for si, t in enumerate(slot_t):
            if t is None:
                continue
            for c in range(KC):
                Xc[:, si, c, :] = xhT[c * 128:(c + 1) * 128, t, :]
        Wc = np.zeros((128, SL, KC, 128), dtype=np.float16)
        Bc = np.zeros((128, UNITS), dtype=np.float32)
        for u, unit in enumerate(sched["units"][core]):
            if unit is None:
                continue
            q, ph = unit
            needed = set(np.flatnonzero(mask[q]).tolist())
            for w in range(WIN[u]):
                si = BASES[u] + w
                t = slot_t[si] if si < NSLOT else None
                if t is not None and t in needed:
                    needed.discard(t)
                    for c in range(KC):
                        Wc[:, OFF[u] + w, c, :] = a3t_block(wh, q, t, ph, c)
            assert not needed, (core, u, unit, needed)
            Bc[:, u] = bias[(ph * 128 + np.arange(128)) * SPA + q]
        Xf = Xc.reshape(128, NSLOT, KC * B)
        Wg = Wc.reshape(128, SL, KC * 128)

        def piece(arr, s0, s1):
            return np.ascontiguousarray(arr[:, s0:s1].reshape(128, -1))

        in_maps.append({
            "Xa": piece(Xf, 0, 3), "Xb": piece(Xf, 3, 5),
            "Xc": piece(Xf, 5, NSLOT),
            "Wa": piece(Wg, 0, 5), "Wb": piece(Wg, 5, 10),
            "Wc": piece(Wg, 10, 15), "We": piece(Wg, 15, 20),
            "Wf": piece(Wg, 20, SL),
            "Bc": Bc,
        })
    return in_maps


def _gather_output(results, sched):
    y = np.zeros((B, C, SPA), dtype=np.float32)
    for core in range(NCORES):
        Yc = results[core]["Yc"].reshape(128, UNITS, B)
        for u, unit in enumerate(sched["units"][core]):
            if unit is None:
                continue
            q, ph = unit
            y[:, ph * 128:(ph + 1) * 128, q] = Yc[:, u, :].T.astype(np.float32)
    return y.reshape(B, OUT)


def _fallback(x, weight, bias, idx):
    a = np.zeros(OUT * IN, dtype=np.float32)
    a[np.asarray(idx, dtype=np.int64)] = weight
    a = a.reshape(OUT, IN)
    return (x @ a.T + bias).astype(np.float32)


def kernel(x, weight, bias, idx):
    global LAST_EXEC_TIME_NS, LAST_RESULT
    x = np.asarray(x, dtype=np.float32)
    weight = np.asarray(weight, dtype=np.float32)
    bias = np.asarray(bias, dtype=np.float32)
    idx = np.asarray(idx)

    mask = _recover_mask(idx)
    sched = None
    if (mask is not None and x.shape == (B, IN)
            and weight.size == mask.sum() * C * C and bias.size == OUT):
        sched = _schedule(mask)
    if sched is None:
        return _fallback(x, weight, bias, idx)

    key = mask.tobytes()
    if key not in _CACHE:
        _CACHE[key] = (sched, _build_program())
    sched, nc = _CACHE[key]

    from concourse.bass_utils import run_bass_kernel_spmd

    in_maps = _prep_inputs(x, weight, bias, sched)
    kwargs = {}
    if TRACE:
        try:
            import profile_hook
            profile_hook.install()
            kwargs["trace"] = True
        except Exception:
            pass
    res = run_bass_kernel_spmd(nc, in_maps, list(range(NCORES)), **kwargs)
    LAST_EXEC_TIME_NS = res.exec_time_ns
    LAST_RESULT = res
    return _gather_output(res.results, sched)


# revision 15
# speedup vs baseline: 1.7051x; 1.0397x over previous
"""Trainium2 Bass kernel for nn_LCNLinear (locally-connected linear layer).

Reference computation:
    a = zeros(4352*4352); a[idx] = weight; a = a.reshape(4352, 4352)
    y = x @ a.T + bias

Structure exploited: idx comes from np.tile(mask17x17, (256, 256)) row-major
flatnonzero, so the scattered matrix dissolves into strided views of the
weight vector and y decomposes into 79 dense 256x256x256 block matmuls
    Y[b, p, q] = sum_{t in band(q)} x[b, s, t] @ A3T[q,t][s, p] + bias
with A3T[q,t] a strided view of weight. No scatter is ever materialized.

Precision: fp16 operands with fp32 PSUM accumulation. Products of two
fp16-rounded operands accumulated in fp32 give ~3e-4 max relative error
(vs the 2e-2 harness gate) at full 1-col/cycle PE rate, 3x less PE work
and ~2x less HBM traffic than a hi+lo split. Outputs are stored fp16
(adds <5e-4) and widened to fp32 on the host.

Sharding (8 cores, SPMD single program): the 34 (q, ph) output groups
(ph = 128-row half of the channel dim) are distributed so each core owns
4-5 groups over a contiguous-ish joint window. Every core runs an
IDENTICAL schedule of 5 PSUM windows sized [5,5,5,5,3] = 23 block-slots;
per-core variation lives only in the data (which weight blocks / which x
t-columns the host stages; unused slots get zero weights). Bias is added
on-device by the DVE copy out of PSUM. A short burst of dummy matmuls on
scratch SBUF runs while the first operands stream in, so the PE's HAM
clock gate is already warm (2.4 GHz) when real work starts.

The host does layout only (shard slicing / transposition / fp16 cast);
all FLOPs and the bias add run on the NeuronCores. If idx is NOT the
bandwidth-2 tiled-mask pattern (it always is for this module), a numpy
fallback computes the reference math directly.
"""

import sys

for _p in ("/opt/trn_rl_repo",):
    if _p not in sys.path:
        sys.path.append(_p)

import numpy as np

SPA = 17
C = 256
B = 256
IN = SPA * C
OUT = SPA * C
NCORES = 8
KC = 2                      # contract chunks of 128 (C = 256)
BW = 2                      # band half-width of the mask
UNITS = 5                   # PSUM windows per core
WIN = [5, 5, 5, 5, 3]       # slots per window
OFF = [0, 5, 10, 15, 20]    # slot offset of each window
BASES = [0, 0, 1, 1, 5]     # X-slot base of each window
SL = 23                     # total W slots per core
NSLOT = 8                   # X t-slots per core

_CACHE = {}

# set by test harness to collect profiling info
TRACE = False
LAST_EXEC_TIME_NS = None
LAST_RESULT = None


def _recover_mask(idx):
    """If idx == flatnonzero(tile(mask, (C, C))) for a 17x17 mask, return the
    boolean mask, else None."""
    idx = np.asarray(idx)
    if idx.ndim != 1 or idx.size == 0 or idx.size % (C * C) != 0:
        return None
    nnzmask = idx.size // (C * C)
    if not 1 <= nnzmask <= SPA * SPA:
        return None
    if idx.min() < 0 or idx.max() >= OUT * IN:
        return None
    q = (idx // IN) % SPA
    t = (idx % IN) % SPA
    mask = np.zeros((SPA, SPA), dtype=bool)
    mask[q, t] = True
    if int(mask.sum()) != nnzmask:
        return None
    idx_rec = np.flatnonzero(np.tile(mask, (C, C)))
    if idx_rec.size != idx.size or not np.array_equal(idx, idx_rec.astype(idx.dtype)):
        return None
    return mask


def _schedule(mask):
    """Static schedule for the |i-j|<=BW band mask. Returns None if the mask
    is not that band (callers then fall back to numpy)."""
    i = np.arange(SPA)
    band = np.abs(i[:, None] - i[None, :]) <= BW
    if not np.array_equal(mask, band):
        return None

    units = []
    slot_t = []
    for core in range(6):          # cores 0..5: q = 2+2c, 3+2c
        qA = 2 + 2 * core
        u = [(qA, 0), (qA, 1), (qA + 1, 0), (qA + 1, 1), None]
        st = [qA - 2, qA - 1, qA, qA + 1, qA + 2, qA + 3, None, None]
        units.append(u)
        slot_t.append(st)
    # core 5 additionally owns (16, 0) in its 5th window
    units[5][4] = (16, 0)
    slot_t[5][6] = 16
    slot_t[5][7] = 14
    # core 6: q14, q15
    units.append([(14, 0), (14, 1), (15, 0), (15, 1), None])
    slot_t.append([12, 13, 14, 15, 16, None, None, None])
    # core 7: q0, q1, (16,1)
    units.append([(0, 0), (0, 1), (1, 0), (1, 1), (16, 1)])
    slot_t.append([0, 1, 2, 3, 0, 14, 15, 16])

    bw = mask.sum(1).astype(int)
    pre = np.concatenate([[0], np.cumsum(bw)[:-1]]).astype(int)
    return {
        "mask": mask, "bw": bw, "pre": pre, "nnzmask": int(bw.sum()),
        "units": units, "slot_t": slot_t,
    }


def _build_program():
    import concourse.tile as tile
    from concourse import bacc, mybir

    f16, f32 = mybir.dt.float16, mybir.dt.float32
    nc = bacc.Bacc("TRN2", target_bir_lowering=False, debug=False,
                   num_devices=NCORES)
    # One DRAM tensor per DMA transfer so each source is a dense block.
    # X pieces: slots [0:3), [3:5), [5:8); W pieces: slots [0:5), [5:10),
    # [10:15), [15:20), [20:23). Layout inside each: [s 128][slot][c][...]
    def dram(name, slots, inner):
        return nc.dram_tensor(name, [128, slots * KC * inner], f16,
                              kind="ExternalInput").ap()

    Xa, Xb, Xc = dram("Xa", 5, B), dram("Xb", 1, B), dram("Xc", 2, B)
    Wa, Wb, Wc = dram("Wa", 5, 128), dram("Wb", 5, 128), dram("Wc", 5, 128)
    We, Wf = dram("We", 5, 128), dram("Wf", 3, 128)
    Bd = nc.dram_tensor("Bc", [128, UNITS], f32, kind="ExternalInput").ap()
    Yd = nc.dram_tensor("Yc", [128, UNITS * B], f16,
                        kind="ExternalOutput").ap()

    with tile.TileContext(nc) as tc:
        with (
            tc.tile_pool(name="xp", bufs=1) as xp,
            tc.tile_pool(name="wp", bufs=1) as wp,
            tc.tile_pool(name="bp", bufs=1) as bp,
            tc.tile_pool(name="op", bufs=1) as op,
            tc.tile_pool(name="wu", bufs=1) as wu,
            tc.tile_pool(name="pp", bufs=1, space="PSUM") as pp,
        ):
            xt = xp.tile([128, NSLOT, KC, B], f16)
            wt = wp.tile([128, SL, KC, 128], f16)
            bt = bp.tile([128, UNITS], f32)
            ot = op.tile([128, UNITS, B], f16)
            dw = wu.tile([128, 128], f16)
            dx = wu.tile([128, 512], f16)

            # PE warm-up: dummy matmuls on zeroed scratch trip the HAM
            # activity window while the real operands stream in, so the
            # first real matmul already runs at 2.4 GHz.
            nc.gpsimd.memset(dw[:], 0)
            nc.gpsimd.memset(dx[:], 0)
            pw = pp.tile([128, 512], f32, tag="warm")
            for _ in range(10):
                nc.tensor.matmul(pw[:], dw[:], dx[:], start=True, stop=True)

            # Reads are balanced across both HWDGE rings in pieces ordered
            # so that each piece lands just before the window needing it;
            # the final pieces on each ring gate only the last window.
            def rx(t, s):
                return t.rearrange("p (s c b) -> p s c b", s=s, c=KC)

            def rw(t, s):
                return t.rearrange("p (s c m) -> p s c m", s=s, c=KC)

            nc.sync.dma_start(xt[:, 0:5], rx(Xa, 5))
            nc.scalar.dma_start(wt[:, 0:5], rw(Wa, 5))
            nc.sync.dma_start(xt[:, 5:6], rx(Xb, 1))
            nc.scalar.dma_start(wt[:, 5:10], rw(Wb, 5))
            nc.sync.dma_start(wt[:, 10:15], rw(Wc, 5))
            nc.scalar.dma_start(wt[:, 15:20], rw(We, 5))
            nc.sync.dma_start(xt[:, 6:NSLOT], rx(Xc, 2))
            nc.scalar.dma_start(wt[:, 20:SL], rw(Wf, 3))
            nc.sync.dma_start(bt[:], Bd)

            for u in range(UNITS):
                ps = pp.tile([128, B], f32, tag=f"ps{u}")
                n = WIN[u] * KC
                k = 0
                for w in range(WIN[u]):
                    si = BASES[u] + w
                    slot = OFF[u] + w
                    for c in range(KC):
                        nc.tensor.matmul(ps[:], wt[:, slot, c, :],
                                         xt[:, si, c, :],
                                         start=(k == 0), stop=(k == n - 1))
                        k += 1
                # out = psum + bias (DVE reads PSUM, writes fp16 SBUF)
                nc.vector.tensor_scalar_add(ot[:, u], ps[:], bt[:, u:u + 1])
                # batched stores: windows 0-3 go out as one 2KB-row DMA,
                # the final 3-slot window alone so the tail is short
                if u == 3:
                    nc.sync.dma_start(Yd[:, 0:4 * B], ot[:, 0:4])
                elif u == 4:
                    nc.sync.dma_start(Yd[:, 4 * B:], ot[:, 4])
    nc.compile()
    return nc


def _prep_inputs(x, weight, bias, sched):
    mask, bw, pre = sched["mask"], sched["bw"], sched["pre"]
    nnzmask = sched["nnzmask"]

    xh = x.astype(np.float16)
    wh = weight.astype(np.float16)
    # [c, t, b] view of x
    xhT = np.ascontiguousarray(xh.reshape(B, C, SPA).transpose(1, 2, 0))

    def a3t_block(src, q, t, ph, c):
        """[128 s, 128 p] strided view of weight array src for block (q,t)."""
        pos = int(np.flatnonzero(mask[q]).tolist().index(t))
        es = src.strides[0]
        view = np.lib.stride_tricks.as_strided(
            src[C * pre[q] + pos:], shape=(C, C),
            strides=(es * int(bw[q]), es * nnzmask * C))
        return view[c * 128:(c + 1) * 128, ph * 128:(ph + 1) * 128]

    in_maps = []
    for core in range(NCORES):
        slot_t = sched["slot_t"][core]
        Xc = np.zeros((128, NSLOT, KC, B), dtype=np.float16)
        for si, t in enumerate(slot_t):
            if t is None:
                continue
            for c in range(KC):
                Xc[:, si, c, :] = xhT[c * 128:(c + 1) * 128, t, :]
        Wc = np.zeros((128, SL, KC, 128), dtype=np.float16)
        Bc = np.zeros((128, UNITS), dtype=np.float32)
        for u, unit in enumerate(sched["units"][core]):
            if unit is None:
                continue
            q, ph = unit
            needed = set(np.flatnonzero(mask[q]).tolist())
            for w in range(WIN[u]):
                si = BASES[u] + w
                t = slot_t[si] if si < NSLOT else None
                if t is not None and t in needed:
                    needed.discard(t)
                    for c in range(KC):
                        Wc[:, OFF[u] + w, c, :] = a3t_block(wh, q, t, ph, c)
            assert not needed, (core, u, unit, needed)
            Bc[:, u] = bias[(ph * 128 + np.arange(128)) * SPA + q]
        Xf = Xc.reshape(128, NSLOT, KC * B)
        Wg = Wc.reshape(128, SL, KC * 128)

        def piece(arr, s0, s1):
            return np.ascontiguousarray(arr[:, s0:s1].reshape(128, -1))

        in_maps.append({
            "Xa": piece(Xf, 0, 5), "Xb": piece(Xf, 5, 6),
            "Xc": piece(Xf, 6, NSLOT),
            "Wa": piece(Wg, 0, 5), "Wb": piece(Wg, 5, 10),
            "Wc": piece(Wg, 10, 15), "We": piece(Wg, 15, 20),
            "Wf": piece(Wg, 20, SL),
            "Bc": Bc,
        })
    return in_maps


def _gather_output(results, sched):
    y = np.zeros((B, C, SPA), dtype=np.float32)
    for core in range(NCORES):
        Yc = results[core]["Yc"].reshape(128, UNITS, B)
        for u, unit in enumerate(sched["units"][core]):
            if unit is None:
                continue
            q, ph = unit
            y[:, ph * 128:(ph + 1) * 128, q] = Yc[:, u, :].T.astype(np.float32)
    return y.reshape(B, OUT)


def _fallback(x, weight, bias, idx):
    a = np.zeros(OUT * IN, dtype=np.float32)
    a[np.asarray(idx, dtype=np.int64)] = weight
    a = a.reshape(OUT, IN)
    return (x @ a.T + bias).astype(np.float32)


def kernel(x, weight, bias, idx):
    global LAST_EXEC_TIME_NS, LAST_RESULT
    x = np.asarray(x, dtype=np.float32)
    weight = np.asarray(weight, dtype=np.float32)
    bias = np.asarray(bias, dtype=np.float32)
    idx = np.asarray(idx)

    mask = _recover_mask(idx)
    sched = None
    if (mask is not None and x.shape == (B, IN)
            and weight.size == mask.sum() * C * C and bias.size == OUT):
        sched = _schedule(mask)
    if sched is None:
        return _fallback(x, weight, bias, idx)

    key = mask.tobytes()
    if key not in _CACHE:
        _CACHE[key] = (sched, _build_program())
    sched, nc = _CACHE[key]

    from concourse.bass_utils import run_bass_kernel_spmd

    in_maps = _prep_inputs(x, weight, bias, sched)
    kwargs = {}
    if TRACE:
        try:
            import profile_hook
            profile_hook.install()
            kwargs["trace"] = True
        except Exception:
            pass
    res = run_bass_kernel_spmd(nc, in_maps, list(range(NCORES)), **kwargs)
    LAST_EXEC_TIME_NS = res.exec_time_ns
    LAST_RESULT = res
    return _gather_output(res.results, sched)
